# revision 35
# baseline (speedup 1.0000x reference)
"""Bass/Trainium2 kernel for nn_Attention_7816840478804 (ragged bag-attention).

Reference computation:
    att[i]   = <x[i], rel_weight[label[i]]>                       # [N]
    e[i]     = softmax of att within each bag (segment)           # [N]
    repre[b] = sum_{i in b} e[i] * x[i] / sum_{i in b} e[i]       # [B, D]
    logits   = repre @ rel_weight.T + bias                        # [B, C]

Key algebraic fusion: matmul distributes over the weighted sum, so
    logits[b] = (sum_i e_i * att_all[i, :]) / (sum_i e_i) + bias
with att_all = x @ rel_weight.T  [N, C].  x is read exactly once (as bf16)
and the bag pooling happens on the tiny [N, 53] matrix.  Softmax
stabilization (max subtraction) is dropped: it cancels exactly, and
|att| < ~10 here so exp() cannot overflow.

Sharding: sentences are split across 8 cores on bag boundaries (2048 bags
per core, host-side searchsorted), padded to a common block count so all
cores run one SPMD graph.  Per-sentence bag-slot indices and the label
one-hot are host-precomputed *data*, keeping the instruction stream static.

Device pipeline per 1024-sentence block:
    DMA xT block -> 12 matmuls (bf16) -> att [53, 1024] PSUM
    -> 8 PE transposes -> [128 sent, 53] tiles
    -> batched DVE: mask by label one-hot, reduce -> att_sel [128, 8]
    -> ACT exp -> e [128, 8]
    -> batched DVE: bag-slot one-hot * e -> Sel [128, 8*w]
    -> 2x4 matmuls (ones|att).T @ Sel accumulate [54, w] PSUM windows
    -> windowed accumulate into [54, 2048+] SBUF
Epilogue: PE-transpose 128-bag chunks, reciprocal, bias; output [2048, 53].
"""

import sys

sys.path.insert(0, "/opt/trn_rl_repo")

import numpy as np

N_CORES = 8
B_TOTAL = 16384
BPC = B_TOTAL // N_CORES  # 2048 bags per core
C = 53
D = 768
NCH = D // 128  # contraction chunks
BLK = 1024  # sentences per block (DMA/batch granularity)
HB = 512  # sentences per pooling window (half block)
TILE = 128
TPB = BLK // TILE  # 8 tiles per block
BAGS_PER_HB = 64  # expected bags per 512-sentence window


# ---------------------------------------------------------------------------
# Host-side packing
# ---------------------------------------------------------------------------

def _pack(x, label, segment_ids, rel_weight, bias):
    """Shard + lay out inputs for the device graph. Returns (in_maps, meta)."""
    import ml_dtypes

    bf = ml_dtypes.bfloat16
    x = np.ascontiguousarray(np.asarray(x, dtype=np.float32))
    label = np.asarray(label).astype(np.int64)
    seg = np.asarray(segment_ids).astype(np.int64)
    rw = np.asarray(rel_weight, dtype=np.float32)
    bs = np.asarray(bias, dtype=np.float32)

    edges = np.searchsorted(seg, np.arange(0, B_TOTAL + 1, BPC), side="left")
    lens = np.diff(edges)
    padn = int(np.ceil(lens.max() / BLK) * BLK)
    nblk = padn // BLK
    nt = padn // TILE

    # slot_raw = seg_local - 64*halfblock; find required window padding
    lo, hi = 0, 0
    per_core = []
    for c in range(N_CORES):
        s, e = int(edges[c]), int(edges[c + 1])
        seg_local = seg[s:e] - c * BPC
        h = np.arange(e - s) // HB
        slot_raw = seg_local - BAGS_PER_HB * h
        if len(slot_raw):
            lo = min(lo, int(slot_raw.min()))
            hi = max(hi, int(slot_raw.max()))
        per_core.append((s, e, slot_raw))
    padb = max(-lo, hi - (BAGS_PER_HB - 1), 8)
    padb = int(np.ceil(padb / 8) * 8)
    w = BAGS_PER_HB + 2 * padb
    assert w <= 512

    iota53 = np.arange(C, dtype=np.float32)
    in_maps = []
    for c in range(N_CORES):
        s, e, slot_raw = per_core[c]
        ln = e - s
        xs = np.zeros((padn, D), dtype=np.float32)
        xs[:ln] = x[s:e]
        # (block, partition=dchunk-row, chunk, col) = x[g*BLK+j, ch*128+p]
        xp = np.ascontiguousarray(
            xs.reshape(nblk, BLK, NCH, 128).transpose(0, 3, 2, 1).astype(bf)
        ).reshape(nblk, 128, NCH * BLK)

        lab = np.zeros(padn, dtype=np.float32)
        lab[:ln] = label[s:e].astype(np.float32)
        lab2 = lab.reshape(nt, TILE)
        # label one-hot, [128, nt*53]: (p, t*53+cc) = 1[label[t*128+p]==cc]
        oh = (lab2[:, :, None] == iota53).astype(bf)
        oh_t = np.ascontiguousarray(oh.transpose(1, 0, 2).reshape(TILE, nt * C))

        slot = np.full(padn, -1.0, dtype=np.float32)
        slot[:ln] = (slot_raw + padb).astype(np.float32)
        assert slot[:ln].min() >= 0 and slot[:ln].max() < w
        slot_t = np.ascontiguousarray(slot.reshape(nt, TILE).T)

        in_maps.append({
            "xin": xp,
            "ohT": oh_t,
            "slotT": slot_t,
            "wtp": np.ascontiguousarray(
                rw.T.reshape(NCH, 128, C).transpose(1, 0, 2).astype(bf)
            ).reshape(128, NCH * C),
            "iotaw": np.tile(np.arange(w, dtype=np.float32), (128, 1)).astype(bf),
            "eye": np.eye(C, dtype=np.float32).astype(bf),
            "eye54": np.eye(C + 1, dtype=np.float32),
            "biasr": np.tile(bs, (128, 1)),
        })

    meta = {"nblk": nblk, "nt": nt, "w": w, "padb": padb, "edges": edges}
    return in_maps, meta


def _numpy_emulate(in_maps, meta):
    """Pure-numpy emulation of the device graph (layout validation)."""
    nblk, w, padb = meta["nblk"], meta["w"], meta["padb"]
    nhb = nblk * BLK // HB
    accw = BAGS_PER_HB * nhb + 2 * padb
    outs = []
    for m in in_maps:
        wt = m["wtp"].astype(np.float32).reshape(128, NCH, C)
        acc = np.zeros((C + 1, accw), dtype=np.float32)
        for g in range(nblk):
            xsb = m["xin"][g].astype(np.float32).reshape(128, NCH, BLK)
            att = np.zeros((C, BLK), dtype=np.float32)
            for ch in range(NCH):
                att += wt[:, ch, :].T @ xsb[:, ch, :]
            for t in range(TPB):
                tg = g * TPB + t
                h = tg // (HB // TILE)
                at = att[:, t * TILE:(t + 1) * TILE].T  # [128, 53]
                at1 = np.concatenate([at, np.ones((TILE, 1), np.float32)], 1)
                oh = m["ohT"][:, tg * C:(tg + 1) * C].astype(np.float32)
                asel = (at * oh).sum(1)
                ev = np.exp(asel)
                sel = (m["iotaw"].astype(np.float32)
                       == m["slotT"][:, tg][:, None]) * ev[:, None]
                acc[:, BAGS_PER_HB * h:BAGS_PER_HB * h + w] += at1.T @ sel
        den = np.maximum(acc[C, padb:padb + BPC], 1e-30)
        outs.append(acc[:C, padb:padb + BPC] / den + m["biasr"][0][:, None])
    return np.concatenate([o.T for o in outs], 0)


# ---------------------------------------------------------------------------
# Device graph
# ---------------------------------------------------------------------------

_GRAPH_CACHE = {}


def _build(nblk, w, padb):
    key = (nblk, w, padb)
    if key in _GRAPH_CACHE:
        return _GRAPH_CACHE[key]

    import concourse.bacc as bacc
    import concourse.bass as bass
    import concourse.mybir as mybir
    from concourse import tile

    f32 = mybir.dt.float32
    bf16 = mybir.dt.bfloat16
    Alu = mybir.AluOpType
    Act = mybir.ActivationFunctionType
    nt = nblk * TPB
    nhb = nblk * BLK // HB
    accw = BAGS_PER_HB * nhb + 2 * padb

    nc = bacc.Bacc("TRN2", target_bir_lowering=False, debug=False)
    xin = nc.dram_tensor("xin", [nblk, 128, NCH * BLK], bf16, kind="ExternalInput").ap()
    ohT = nc.dram_tensor("ohT", [128, nt * C], bf16, kind="ExternalInput").ap()
    slotT = nc.dram_tensor("slotT", [128, nt], f32, kind="ExternalInput").ap()
    wtp = nc.dram_tensor("wtp", [128, NCH * C], bf16, kind="ExternalInput").ap()
    iotaw = nc.dram_tensor("iotaw", [128, w], bf16, kind="ExternalInput").ap()
    eye = nc.dram_tensor("eye", [C, C], bf16, kind="ExternalInput").ap()
    eye54 = nc.dram_tensor("eye54", [C + 1, C + 1], f32, kind="ExternalInput").ap()
    biasr = nc.dram_tensor("biasr", [128, C], f32, kind="ExternalInput").ap()
    out_t = nc.dram_tensor("out", [BPC, C], f32, kind="ExternalOutput").ap()

    def rep_mid(ap, n):
        return bass.AP(ap.tensor, ap.offset, [ap.ap[0], [0, n], ap.ap[1]])

    def rep_last(ap, n):
        return bass.AP(ap.tensor, ap.offset, [ap.ap[0], ap.ap[1], [0, n]])

    with tile.TileContext(nc) as tc:
        with (
            tc.tile_pool(name="const", bufs=1) as cpool,
            tc.tile_pool(name="accp", bufs=1) as accpool,
            tc.tile_pool(name="xp", bufs=6) as xpool,
            tc.tile_pool(name="ohp", bufs=3) as ohpool,
            tc.tile_pool(name="attp", bufs=3) as apool,
            tc.tile_pool(name="small", bufs=6) as spool,
            tc.tile_pool(name="ep", bufs=3) as epool,
            tc.tile_pool(name="ps_att", bufs=2, space="PSUM") as ps_att,
            tc.tile_pool(name="ps_tr", bufs=3, space="PSUM") as ps_tr,
            tc.tile_pool(name="ps_num", bufs=3, space="PSUM") as ps_num,
        ):
            wt_sb = cpool.tile([128, NCH * C], bf16, tag="wt")
            nc.sync.dma_start(wt_sb, wtp)
            iotaw_sb = cpool.tile([128, w], bf16, tag="iw")
            nc.sync.dma_start(iotaw_sb, iotaw)
            eye_sb = cpool.tile([C, C], bf16, tag="eye")
            nc.sync.dma_start(eye_sb, eye)
            eye54_sb = cpool.tile([C + 1, C + 1], f32, tag="eye54")
            nc.sync.dma_start(eye54_sb, eye54)
            biasr_sb = cpool.tile([128, C], f32, tag="biasr")
            nc.sync.dma_start(biasr_sb, biasr)
            slotT_sb = cpool.tile([128, nt], f32, tag="slot")
            nc.sync.dma_start(slotT_sb, slotT)

            acc = accpool.tile([C + 1, accw], f32, tag="acc")
            nc.vector.memset(acc, 0.0)

            for g in range(nblk):
                x_sb = xpool.tile([128, NCH * BLK], bf16, tag="x")
                nc.sync.dma_start(x_sb, xin[g])
                oh_sb = ohpool.tile([128, TPB * C], bf16, tag="oh")
                nc.sync.dma_start(oh_sb, ohT[:, g * TPB * C:(g + 1) * TPB * C])

                att_row = apool.tile([C, BLK], bf16, tag="attrow")
                for half in range(2):
                    aps = ps_att.tile([C, HB], f32, tag="aps")
                    for ch in range(NCH):
                        nc.tensor.matmul(
                            aps,
                            wt_sb[:, ch * C:(ch + 1) * C],
                            x_sb[:, ch * BLK + half * HB:ch * BLK + (half + 1) * HB],
                            start=(ch == 0),
                            stop=(ch == NCH - 1),
                        )
                    nc.scalar.copy(att_row[:, half * HB:(half + 1) * HB], aps)

                # 8 transposes into one psum tile, 54-el stride (4B aligned)
                trp = ps_tr.tile([128, TPB * (C + 1)], bf16, tag="trp")
                trp_v = bass.AP(
                    trp.tensor, trp.offset, [trp.ap[0], [C + 1, TPB], [1, C]]
                )
                for t in range(TPB):
                    nc.tensor.transpose(
                        trp[:, t * (C + 1):t * (C + 1) + C],
                        att_row[:, t * TILE:(t + 1) * TILE],
                        eye_sb,
                    )
                # at1: 8 x [att(53) | ones] laid out [128, 8*54]
                at1 = spool.tile([128, TPB * (C + 1)], bf16, tag="at1")
                at1_att = bass.AP(
                    at1.tensor, at1.offset, [at1.ap[0], [C + 1, TPB], [1, C]]
                )
                nc.scalar.copy(at1_att, trp_v)
                at1_ones = bass.AP(
                    at1.tensor, at1.offset + C, [at1.ap[0], [C + 1, TPB]]
                )
                nc.vector.memset(at1_ones, 1.0)

                col = slice(g * TPB, g * TPB + TPB)
                # mask att by label one-hot, per-tile reduce -> asel [128,8]
                mk = spool.tile([128, TPB * C], bf16, tag="mk")
                nc.vector.scalar_tensor_tensor(
                    mk.rearrange("p (t c) -> p t c", t=TPB),
                    oh_sb.rearrange("p (t c) -> p t c", t=TPB),
                    0.0,
                    at1_att,
                    Alu.bypass,
                    Alu.mult,
                )
                asel = spool.tile([128, TPB], f32, tag="asel")
                nc.vector.tensor_reduce(
                    asel,
                    mk.rearrange("p (t c) -> p t c", t=TPB),
                    mybir.AxisListType.X,
                    Alu.add,
                )
                ev = spool.tile([128, TPB], f32, tag="ev")
                nc.scalar.activation(ev, asel, Act.Exp)

                # bag one-hot * e -> sel [128, 8*w]
                ohw = spool.tile([128, TPB * w], bf16, tag="ohw")
                nc.vector.scalar_tensor_tensor(
                    ohw.rearrange("p (t v) -> p t v", t=TPB),
                    rep_mid(iotaw_sb, TPB),
                    0.0,
                    rep_last(slotT_sb[:, col], w),
                    Alu.bypass,
                    Alu.is_equal,
                )
                sel = spool.tile([128, TPB * w], bf16, tag="sel")
                nc.vector.scalar_tensor_tensor(
                    sel.rearrange("p (t v) -> p t v", t=TPB),
                    ohw.rearrange("p (t v) -> p t v", t=TPB),
                    0.0,
                    rep_last(ev, w),
                    Alu.bypass,
                    Alu.mult,
                )

                for half in range(2):
                    nps = ps_num.tile([C + 1, w], f32, tag="nps")
                    for t4 in range(4):
                        t = half * 4 + t4
                        nc.tensor.matmul(
                            nps,
                            at1[:, t * (C + 1):(t + 1) * (C + 1)],
                            sel[:, t * w:(t + 1) * w],
                            start=(t4 == 0),
                            stop=(t4 == 3),
                        )
                    off = BAGS_PER_HB * (2 * g + half)
                    nc.vector.scalar_tensor_tensor(
                        acc[:, off:off + w],
                        nps,
                        0.0,
                        acc[:, off:off + w],
                        Alu.bypass,
                        Alu.add,
                    )

            # epilogue: transpose 128-bag chunks, divide, add bias
            for p in range(BPC // TILE):
                tps = ps_att.tile([128, C + 1], f32, tag="aps")
                nc.tensor.transpose(
                    tps, acc[:, padb + p * TILE:padb + (p + 1) * TILE], eye54_sb
                )
                den1 = epool.tile([128, 1], f32, tag="den1")
                nc.vector.tensor_scalar(den1, tps[:, C:C + 1], 1e-30, None, Alu.max)
                rec1 = epool.tile([128, 1], f32, tag="rec1")
                nc.vector.reciprocal(rec1, den1)
                logb = epool.tile([128, C], f32, tag="logb")
                nc.vector.scalar_tensor_tensor(
                    logb, tps[:, 0:C], rec1, biasr_sb, Alu.mult, Alu.add
                )
                nc.sync.dma_start(out_t[p * TILE:(p + 1) * TILE, :], logb)

    nc.compile()
    _GRAPH_CACHE[key] = nc
    return nc


# ---------------------------------------------------------------------------
# Entry point
# ---------------------------------------------------------------------------

_last_results = None


def _install_ntff_hook():
    """Provide antenv.axon_hooks (missing in this image) from trn_boot."""
    try:
        from antenv import axon_hooks  # noqa: F401
        return
    except ImportError:
        pass
    import types

    import antenv
    from trn_agent_boot.trn_boot import _ntff_profile_via_ctypes

    hook = _ntff_profile_via_ctypes("/opt/axon/libaxon_pjrt.so")
    m = types.ModuleType("antenv.axon_hooks")
    m.get_axon_ntff_profile_hook = lambda: hook
    m.set_axon_ntff_profile_hook = lambda h: None
    sys.modules["antenv.axon_hooks"] = m
    antenv.axon_hooks = m


def kernel(x, label, segment_ids, rel_weight, bias):
    import concourse.bass_utils as bu
    from concourse.bass_utils import run_bass_kernel_spmd

    in_maps, meta = _pack(x, label, segment_ids, rel_weight, bias)
    nc = _build(meta["nblk"], meta["w"], meta["padb"])

    global _last_results
    import os

    trace = bool(os.environ.get("KERNEL_TRACE"))
    tmpdir = None
    if trace:
        _install_ntff_hook()
        bu.upload_artifacts = lambda d: d  # no bucket in this container
        tmpdir = os.environ.get("KERNEL_TRACE_DIR")
    res = run_bass_kernel_spmd(
        nc, in_maps, core_ids=list(range(N_CORES)), trace=trace, tmpdir=tmpdir
    )
    _last_results = res
    out = np.empty((B_TOTAL, C), dtype=np.float32)
    for c in range(N_CORES):
        out[c * BPC:(c + 1) * BPC] = res.results[c]["out"]
    return out
